# revision 19
# baseline (speedup 1.0000x reference)
"""RGGB demosaic (reflect-pad + 4x 5x5 conv + parity scatter + clip) on 8 TRN2 cores.

Pure data parallel: host reflect-pads and column-DEINTERLEAVES the mosaic into
even/odd column planes [xe | xo] (4, 1028, 2052) bf16, sharded (image, H-half)
across 8 cores.

Per output pixel the reference either copies the mosaic value x (4 of the 12
(channel, row-parity, col-parity) combinations) or evaluates one of four 5x5
convs. The device computes ONLY the conv sites; the host scatters x into the
copy sites for free and interleaves the conv planes.

Device compute: the conv runs on the TensorEngine as banded-matrix matmuls.
A 128x128 stationary matrix W applies arbitrary per-output-row vertical
5-tap filters to a 128-row input block. Horizontal symmetry of all four
kernels (col0==col4, col1==col3) collapses the 5 horizontal taps into 3
accumulating passes over three movers per column parity p:
  center   x(J)              = plane slice (no compute)
  u_p(j) = x(J-1)+x(J+1)     adjacent cols, opposite-parity plane
  w_p(j) = x(J-2)+x(J+2)     same-parity plane, +-1 index
Because the host deinterleaved the planes, ALL movers are contiguous
stride-1 slices. Each matmul computes TWO quarter-resolution fields at
once: output partition m < 64 is quarter-field A row 2m+ra, partition
64+m is quarter-field B -- row-parity subsampling and 2x row packing are
both encoded in W. u_p (one odd-offset operand, which VectorE faults on)
runs on GpSimd; w_p (4B-aligned) runs on VectorE. All PSUM evacuations
run on the ACT engine (Relu + cast): PSUM reads don't contend with the
PE's SBUF moving-tensor stream, which slows SBUF-sourced vector ops ~3x
while the PE streams. The 9th (tail) block only contributes quarter-rows
496..511, so a packed set of W matrices computes all four (pack, slot)
fields x 16 rows in 64 output partitions -- 12 matmuls instead of 24.
bf16 throughout the PE path: all tap values are dyadic rationals,
accumulation in fp32 PSUM; rel err ~1e-3 vs the fp32 reference.
"""

import numpy as np

_NCORES = 8
_H = 2048
_W = 2048
_N = 4
_HH = _H // 2  # rows per core (1024)
_PW = _W + 4  # padded width (2052 = 2x1026 plane columns)
_PLANE = _PW // 2  # 1026 columns per parity plane
_BLK_OUT = 124  # valid output rows per 128-row block
_NBLK = 9  # 8 full strides + 1 tail block
_CHUNK = 512  # psum free size (one bank, fp32)
_NCHUNK = 2  # 2 chunks x 512 half-res cols = 1024 = W/2

# (colparity p, pack, slot) -> (channel, row parity ra, kernel index)
# kernel index: 0=kgrb, 1=krbg0, 2=krbg1, 3=krbbr
_QF_MAP = {
    (0, 0, 0): (1, 0, 0),  # G[0::2,0::2] = kgrb
    (0, 0, 1): (2, 0, 3),  # B[0::2,0::2] = krbbr
    (0, 1, 0): (0, 1, 2),  # R[1::2,0::2] = krbg1
    (0, 1, 1): (2, 1, 1),  # B[1::2,0::2] = krbg0
    (1, 0, 0): (0, 0, 1),  # R[0::2,1::2] = krbg0
    (1, 0, 1): (2, 0, 2),  # B[0::2,1::2] = krbg1
    (1, 1, 0): (1, 1, 0),  # G[1::2,1::2] = kgrb
    (1, 1, 1): (0, 1, 3),  # R[1::2,1::2] = krbbr
}


def _demosaic_kernels():
    kgrb = 1 / 8 * np.array(
        [[0, 0, -1, 0, 0], [0, 0, 2, 0, 0], [-1, 2, 4, 2, -1], [0, 0, 2, 0, 0], [0, 0, -1, 0, 0]],
        dtype=np.float64)
    krbg0 = 1 / 8 * np.array(
        [[0, 0, 0.5, 0, 0], [0, -1, 0, -1, 0], [-1, 4, 5, 4, -1], [0, -1, 0, -1, 0], [0, 0, 0.5, 0, 0]],
        dtype=np.float64)
    krbg1 = krbg0.T.copy()
    krbbr = 1 / 8 * np.array(
        [[0, 0, -1.5, 0, 0], [0, 2, 0, 2, 0], [-1.5, 0, 6, 0, -1.5], [0, 2, 0, 2, 0], [0, 0, -1.5, 0, 0]],
        dtype=np.float64)
    return [kgrb, krbg0, krbg1, krbbr]


def _build_wmats():
    """12 stationary 128x128 matrices: [p][pack][mover] with movers (x,u,w)
    taking kernel columns 2,1,0 as vertical tap vectors. Output partition
    m<64 is quarter-field slot 0 (rows 2m+ra), partition 64+m slot 1:
    W[2m+ra+d, 64*slot+m] = K[d, col]. Block starts are even, so relative
    row parity == absolute row parity."""
    ks = _demosaic_kernels()
    for K in ks:
        assert np.allclose(K[:, 0], K[:, 4]) and np.allclose(K[:, 1], K[:, 3])
    wm = np.zeros((128, 18 * 128), dtype=np.float64)
    for p in range(2):
        for pack in range(2):
            for vi, col in enumerate((2, 1, 0)):
                W = np.zeros((128, 128))
                for slot in range(2):
                    _, ra, ki = _QF_MAP[(p, pack, slot)]
                    K = ks[ki]
                    for m in range(62):
                        for d in range(5):
                            W[2 * m + ra + d, 64 * slot + m] = K[d, col]
                idx = (p * 2 + pack) * 3 + vi
                wm[:, idx * 128:(idx + 1) * 128] = W
    # tail-block matrices (idx 12..17, 64 used cols): all four (pack, slot)
    # quarter-fields packed 16 rows each (m=46..61, the only rows of the tail
    # block the host consumes) -> the tail needs 12 matmuls instead of 24
    for p in range(2):
        for vi, col in enumerate((2, 1, 0)):
            W = np.zeros((128, 128))
            for pack in range(2):
                for slot in range(2):
                    _, ra, ki = _QF_MAP[(p, pack, slot)]
                    K = ks[ki]
                    for m in range(46, 62):
                        for d in range(5):
                            W[2 * m + ra + d, 16 * (2 * pack + slot) + m - 46] = K[d, col]
            idx = 12 + p * 3 + vi
            wm[:, idx * 128:(idx + 1) * 128] = W
    import ml_dtypes
    return wm.astype(ml_dtypes.bfloat16)


def _patch_tile_drain():
    """This container's walrus build fits only ONE sem-wait command per
    instruction (any type); Tile freely attaches several and codegen dies
    with 'Too many sync wait commands'. Two patches:
      1. _commit_and_lower: before an over-waited instruction lands in its
         basic block, emit nofuse nops on the same engine carrying the
         excess waits (waiting earlier on the same engine is equivalent).
      2. the exit drain: split its per-proc-lane waits across a drain chain
         (all still complete before the barrier + sem clear).
    """
    import concourse.tile as _tile
    import concourse.mybir as mybir
    from concourse.vector_clock import ScopedClock

    if getattr(_tile.TileContext, "_demosaic_drain_patch", False):
        return

    _orig_commit_and_lower = _tile.TileContext._commit_and_lower

    def _commit_and_lower(self, inst, original_block, old_bb_map, bb_to_exit_bb):
        si = getattr(inst, "sync_info", None)
        eng = getattr(inst, "engine", None)
        if (
            si is not None
            and si.on_wait
            and len(si.on_wait) > 1
            and eng is not None
            and eng in self.nc.engines
        ):
            waits = list(si.on_wait)
            for w in waits[:-1]:
                nop = self.nc.engines[eng].nop(nofuse=True, hint="waitsplit")
                nop.ins.sync_info = mybir.SyncInfo(on_wait=[w], on_update=[])
            inst.sync_info = mybir.SyncInfo(
                on_wait=waits[-1:], on_update=list(si.on_update or [])
            )
        return _orig_commit_and_lower(
            self, inst, original_block, old_bb_map, bb_to_exit_bb
        )

    def _drain_and_barrier(self, tick_clock, wait_clock):
        nc = self.nc
        drain_inst = nc.sync.drain()
        wait_clock.add_sem_waits(
            drain_inst.ins, ScopedClock({None: tick_clock.global_clock})
        )
        si = drain_inst.ins.sync_info
        waits = list(si.on_wait) if si and si.on_wait else []
        if len(waits) > 1:
            si.on_wait = waits[:1]
            for i in range(1, len(waits)):
                extra = nc.sync.drain()
                extra.ins.sync_info = mybir.SyncInfo(
                    on_wait=waits[i:i + 1], on_update=[]
                )
        nc.all_engine_barrier()
        assert self.sems is not None
        popped = nc._tile_sem_poison_stack.pop()
        assert popped is self._sem_poison
        nc.clear_and_free_semaphores(list(self.sems.allocated().values()))
        nc.all_engine_barrier()

    _tile.TileContext._commit_and_lower = _commit_and_lower
    _tile.TileContext._drain_and_barrier = _drain_and_barrier
    _tile.TileContext._demosaic_drain_patch = True


def _build_bass():
    _patch_tile_drain()
    import concourse.bass as bass
    import concourse.mybir as mybir
    import concourse.tile as tile

    f32 = mybir.dt.float32
    f16 = mybir.dt.bfloat16
    Alu = mybir.AluOpType
    Act = mybir.ActivationFunctionType

    nc = bass.Bass()
    # input: [xe | xo] deinterleaved planes, 1026 cols each
    xp = nc.dram_tensor("xp", (_HH + 4, _PW), f16, kind="ExternalInput")
    # compact conv-site output, full blocks: [block, 128 rows, (pp,pack)*1024]
    qout = nc.dram_tensor("qout", (8, 128, 4 * (_W // 2)), f16,
                          kind="ExternalOutput")
    # tail block: 4 quarter-fields x 16 rows packed in 64 partitions, pp-major
    qtail = nc.dram_tensor("qtail", (2, 64, _W // 2), f16, kind="ExternalOutput")
    wdram = nc.inline_tensor(_build_wmats(), name="wmats")

    XO = _PLANE  # xo plane base column in the packed tile

    with tile.TileContext(nc) as tc:
        with (
            tc.tile_pool(name="wpool", bufs=1) as wpool,
            tc.tile_pool(name="xpool", bufs=3) as xpool,
            tc.tile_pool(name="uwpool", bufs=3) as uwpool,
            tc.tile_pool(name="qpool", bufs=2) as qpool,
            tc.tile_pool(name="psum", bufs=4, space="PSUM") as psum_pool,
        ):
            wt = wpool.tile([128, 18 * 128], f16)

            for b in range(_NBLK):
                s0 = 124 * b if b < 8 else (_HH - _BLK_OUT)  # block start (padded row)

                xt = xpool.tile([128, _PW], f16)
                if b == 0:
                    # stream-start critical path: xo half first (feeds the
                    # GpSimd u0 add immediately), W overlapping u0's compute,
                    # xe half last (gates only the x-pass matmuls)
                    nc.sync.dma_start(xt[:, XO:_PW], xp[s0:s0 + 128, XO:_PW])
                    nc.sync.dma_start(wt[:], wdram[:])
                    nc.sync.dma_start(xt[:, 0:XO], xp[s0:s0 + 128, 0:XO])
                else:
                    nc.sync.dma_start(xt[:], xp[s0:s0 + 128, :])

                # movers, half-res col j in 0..1023 (full-res J = 2j+p+2):
                #   p=0: center xe[j+1];  u0 = xo[j]+xo[j+1];  w0 = xe[j]+xe[j+2]
                #   p=1: center xo[j+1];  u1 = xe[j+1]+xe[j+2]; w1 = xo[j]+xo[j+2]
                # u operands include an odd element offset (VectorE faults) ->
                # GpSimd; w operands are 4B-aligned -> VectorE. Separate tiles
                # per mover keep producer->consumer dependencies narrow (a
                # fused tile serializes the first matmul behind all four TTs).
                u0 = uwpool.tile([128, 1024], f16, name=f"u0_b{b}", tag="u0")
                u1 = uwpool.tile([128, 1024], f16, name=f"u1_b{b}", tag="u1")
                w0 = uwpool.tile([128, 1024], f16, name=f"w0_b{b}", tag="w0")
                w1 = uwpool.tile([128, 1024], f16, name=f"w1_b{b}", tag="w1")
                nc.gpsimd.tensor_tensor(
                    u0[:], xt[:, XO:XO + 1024], xt[:, XO + 1:XO + 1025], Alu.add)
                nc.vector.tensor_tensor(
                    w0[:], xt[:, 0:1024], xt[:, 2:1026], Alu.add)
                nc.gpsimd.tensor_tensor(
                    u1[:], xt[:, 1:1025], xt[:, 2:1026], Alu.add)
                nc.vector.tensor_tensor(
                    w1[:], xt[:, XO:XO + 1024], xt[:, XO + 2:XO + 1026], Alu.add)

                movers = {
                    (0, 0): xt[:, 1:1025],
                    (0, 1): u0[:],
                    (0, 2): w0[:],
                    (1, 0): xt[:, XO + 1:XO + 1025],
                    (1, 1): u1[:],
                    (1, 2): w1[:],
                }

                if b < 8:
                    for pp in range(2):
                        for pack in range(2):
                            qt = qpool.tile([128, 1024], f16,
                                            name=f"qt{pp}{pack}_b{b}", tag=f"q{pp}{pack}")
                            ps = psum_pool.tile([128, 1024], f32,
                                                name=f"ps_b{b}p{pp}{pack}", tag="ps")
                            for vi in range(3):
                                mv = movers[(pp, vi)]
                                idx = (pp * 2 + pack) * 3 + vi
                                for c in range(_NCHUNK):
                                    nc.tensor.matmul(
                                        ps[:, 512 * c:512 * c + 512],
                                        wt[:, idx * 128:(idx + 1) * 128],
                                        mv[:, 512 * c:512 * c + 512],
                                        start=(vi == 0),
                                        stop=(vi == 2),
                                    )
                            # all evacuations on ACT: PSUM reads don't contend
                            # with the PE's SBUF moving-tensor stream (SBUF-
                            # sourced vector ops run ~3x slower under PE load).
                            # Relu only; the host applies min(.,1).
                            e = (pp * 2 + pack) * 1024
                            nc.scalar.activation(qt[:], ps[:], Act.Relu, bias=0.0)
                            nc.sync.dma_start(qout[b, :, e:e + 1024], qt[:])
                else:
                    # tail block: the host only consumes quarter-rows 496..511
                    # (m=46..61); a packed W computes all four (pack, slot)
                    # fields x 16 rows in 64 output partitions -> 12 matmuls
                    # instead of 24 and a shorter post-stream drain
                    for pp in range(2):
                        qt = qpool.tile([64, 1024], f16, name=f"qtail{pp}",
                                        tag=f"qtl{pp}")
                        ps = psum_pool.tile([128, 1024], f32, name=f"ps_tp{pp}",
                                            tag="ps")
                        for vi in range(3):
                            mv = movers[(pp, vi)]
                            idx = 12 + pp * 3 + vi
                            for c in range(_NCHUNK):
                                nc.tensor.matmul(
                                    ps[0:64, 512 * c:512 * c + 512],
                                    wt[:, idx * 128:idx * 128 + 64],
                                    mv[:, 512 * c:512 * c + 512],
                                    start=(vi == 0),
                                    stop=(vi == 2),
                                )
                        nc.scalar.activation(qt[:], ps[0:64, 0:1024], Act.Relu,
                                             bias=0.0)
                        nc.sync.dma_start(qtail[pp], qt[:])

    return nc


_nc_cache = None

# set by an external harness to capture an NTFF profile; harmless otherwise
TRACE = False
LAST_EXEC_NS = None
LAST_RESULT = None


def _get_nc():
    global _nc_cache
    if _nc_cache is None:
        _nc_cache = _build_bass()
    return _nc_cache


def kernel(x: np.ndarray, k: np.ndarray | None = None) -> np.ndarray:
    """x: (4, 1, 2048, 2048) float32 RGGB mosaic -> (4, 3, 2048, 2048) float32."""
    from concourse.bass_utils import run_bass_kernel_spmd

    x = np.asarray(x, dtype=np.float32)
    assert x.shape == (_N, 1, _H, _W), x.shape
    import ml_dtypes
    xpad = np.pad(x[:, 0], ((0, 0), (2, 2), (2, 2)), mode="reflect")
    # deinterleave columns: [xe | xo] planes, then bf16
    xdi = np.concatenate([xpad[:, :, 0::2], xpad[:, :, 1::2]], axis=2)
    xdi = xdi.astype(ml_dtypes.bfloat16)

    in_maps = []
    for core in range(_NCORES):
        img, half = divmod(core, 2)
        h0 = half * _HH
        in_maps.append({"xp": np.ascontiguousarray(xdi[img, h0:h0 + _HH + 4, :])})

    nc = _get_nc()
    res = run_bass_kernel_spmd(nc, in_maps, list(range(_NCORES)), trace=TRACE)
    global LAST_EXEC_NS, LAST_RESULT
    LAST_EXEC_NS = res.exec_time_ns
    LAST_RESULT = res

    outp = np.empty((_N, 3, _H, _W), dtype=np.float32)
    for core in range(_NCORES):
        img, half = divmod(core, 2)
        h0 = half * _HH
        xs = x[img, 0, h0:h0 + _HH, :]
        o = outp[img, :, h0:h0 + _HH, :]
        # copy sites straight from the mosaic (exact, already in [0,1))
        o[0, 0::2, 0::2] = xs[0::2, 0::2]
        o[1, 0::2, 1::2] = xs[0::2, 1::2]
        o[1, 1::2, 0::2] = xs[1::2, 0::2]
        o[2, 1::2, 1::2] = xs[1::2, 1::2]
        # conv sites from the device's compact quarter-fields
        q = res.results[core]["qout"]  # (8, 128, 4096) fp16
        qt = res.results[core]["qtail"]  # (2, 64, 1024) fp16
        for (pp, pack, slot), (ch, ra, _ki) in _QF_MAP.items():
            Q = np.empty((_HH // 2, _W // 2), dtype=np.float32)
            e = (pp * 2 + pack) * 1024
            rows = q[:, 64 * slot:64 * slot + 64, e:e + 1024]
            for b in range(8):
                Q[62 * b:62 * b + 62] = rows[b, 0:62]
            g = 16 * (2 * pack + slot)
            Q[496:512] = qt[pp, g:g + 16]
            o[ch, ra::2, pp::2] = np.minimum(Q, 1.0)
    return outp


# revision 20
# speedup vs baseline: 1.0492x; 1.0492x over previous
"""RGGB demosaic (reflect-pad + 4x 5x5 conv + parity scatter + clip) on 8 TRN2 cores.

Pure data parallel: host reflect-pads and column-DEINTERLEAVES the mosaic into
even/odd column planes [xe | xo] (4, 1028, 2052) bf16, sharded (image, H-half)
across 8 cores.

Per output pixel the reference either copies the mosaic value x (4 of the 12
(channel, row-parity, col-parity) combinations) or evaluates one of four 5x5
convs. The device computes ONLY the conv sites; the host scatters x into the
copy sites for free and interleaves the conv planes.

Device compute: the conv runs on the TensorEngine as banded-matrix matmuls.
A 128x128 stationary matrix W applies arbitrary per-output-row vertical
5-tap filters to a 128-row input block. Horizontal symmetry of all four
kernels (col0==col4, col1==col3) collapses the 5 horizontal taps into 3
accumulating passes over three movers per column parity p:
  center   x(J)              = plane slice (no compute)
  u_p(j) = x(J-1)+x(J+1)     adjacent cols, opposite-parity plane
  w_p(j) = x(J-2)+x(J+2)     same-parity plane, +-1 index
Because the host deinterleaved the planes, ALL movers are contiguous
stride-1 slices. Each matmul computes TWO quarter-resolution fields at
once: output partition m < 64 is quarter-field A row 2m+ra, partition
64+m is quarter-field B -- row-parity subsampling and 2x row packing are
both encoded in W. u_p (one odd-offset operand, which VectorE faults on)
runs on GpSimd; w_p (4B-aligned) runs on VectorE. All PSUM evacuations
run on the ACT engine (Relu + cast): PSUM reads don't contend with the
PE's SBUF moving-tensor stream, which slows SBUF-sourced vector ops ~3x
while the PE streams. The 9th (tail) block only contributes quarter-rows
496..511, so a packed set of W matrices computes all four (pack, slot)
fields x 16 rows in 64 output partitions -- 12 matmuls instead of 24.
bf16 throughout the PE path: all tap values are dyadic rationals,
accumulation in fp32 PSUM; rel err ~1e-3 vs the fp32 reference.
"""

import numpy as np

_NCORES = 8
_H = 2048
_W = 2048
_N = 4
_HH = _H // 2  # rows per core (1024)
_PW = _W + 4  # padded width (2052 = 2x1026 plane columns)
_PLANE = _PW // 2  # 1026 columns per parity plane
_BLK_OUT = 124  # valid output rows per 128-row block
_NBLK = 9  # 8 full strides + 1 tail block
_CHUNK = 512  # psum free size (one bank, fp32)
_NCHUNK = 2  # 2 chunks x 512 half-res cols = 1024 = W/2

# (colparity p, pack, slot) -> (channel, row parity ra, kernel index)
# kernel index: 0=kgrb, 1=krbg0, 2=krbg1, 3=krbbr
_QF_MAP = {
    (0, 0, 0): (1, 0, 0),  # G[0::2,0::2] = kgrb
    (0, 0, 1): (2, 0, 3),  # B[0::2,0::2] = krbbr
    (0, 1, 0): (0, 1, 2),  # R[1::2,0::2] = krbg1
    (0, 1, 1): (2, 1, 1),  # B[1::2,0::2] = krbg0
    (1, 0, 0): (0, 0, 1),  # R[0::2,1::2] = krbg0
    (1, 0, 1): (2, 0, 2),  # B[0::2,1::2] = krbg1
    (1, 1, 0): (1, 1, 0),  # G[1::2,1::2] = kgrb
    (1, 1, 1): (0, 1, 3),  # R[1::2,1::2] = krbbr
}


def _demosaic_kernels():
    kgrb = 1 / 8 * np.array(
        [[0, 0, -1, 0, 0], [0, 0, 2, 0, 0], [-1, 2, 4, 2, -1], [0, 0, 2, 0, 0], [0, 0, -1, 0, 0]],
        dtype=np.float64)
    krbg0 = 1 / 8 * np.array(
        [[0, 0, 0.5, 0, 0], [0, -1, 0, -1, 0], [-1, 4, 5, 4, -1], [0, -1, 0, -1, 0], [0, 0, 0.5, 0, 0]],
        dtype=np.float64)
    krbg1 = krbg0.T.copy()
    krbbr = 1 / 8 * np.array(
        [[0, 0, -1.5, 0, 0], [0, 2, 0, 2, 0], [-1.5, 0, 6, 0, -1.5], [0, 2, 0, 2, 0], [0, 0, -1.5, 0, 0]],
        dtype=np.float64)
    return [kgrb, krbg0, krbg1, krbbr]


def _build_wmats():
    """12 stationary 128x128 matrices: [p][pack][mover] with movers (x,u,w)
    taking kernel columns 2,1,0 as vertical tap vectors. Output partition
    m<64 is quarter-field slot 0 (rows 2m+ra), partition 64+m slot 1:
    W[2m+ra+d, 64*slot+m] = K[d, col]. Block starts are even, so relative
    row parity == absolute row parity."""
    ks = _demosaic_kernels()
    for K in ks:
        assert np.allclose(K[:, 0], K[:, 4]) and np.allclose(K[:, 1], K[:, 3])
    wm = np.zeros((128, 18 * 128), dtype=np.float64)
    for p in range(2):
        for pack in range(2):
            for vi, col in enumerate((2, 1, 0)):
                W = np.zeros((128, 128))
                for slot in range(2):
                    _, ra, ki = _QF_MAP[(p, pack, slot)]
                    K = ks[ki]
                    for m in range(62):
                        for d in range(5):
                            W[2 * m + ra + d, 64 * slot + m] = K[d, col]
                idx = (p * 2 + pack) * 3 + vi
                wm[:, idx * 128:(idx + 1) * 128] = W
    # tail-block matrices (idx 12..17, 64 used cols): all four (pack, slot)
    # quarter-fields packed 16 rows each (m=46..61, the only rows of the tail
    # block the host consumes) -> the tail needs 12 matmuls instead of 24
    for p in range(2):
        for vi, col in enumerate((2, 1, 0)):
            W = np.zeros((128, 128))
            for pack in range(2):
                for slot in range(2):
                    _, ra, ki = _QF_MAP[(p, pack, slot)]
                    K = ks[ki]
                    for m in range(46, 62):
                        for d in range(5):
                            W[2 * m + ra + d, 16 * (2 * pack + slot) + m - 46] = K[d, col]
            idx = 12 + p * 3 + vi
            wm[:, idx * 128:(idx + 1) * 128] = W
    import ml_dtypes
    return wm.astype(ml_dtypes.bfloat16)


def _patch_tile_drain():
    """This container's walrus build fits only ONE sem-wait command per
    instruction (any type); Tile freely attaches several and codegen dies
    with 'Too many sync wait commands'. Two patches:
      1. _commit_and_lower: before an over-waited instruction lands in its
         basic block, emit nofuse nops on the same engine carrying the
         excess waits (waiting earlier on the same engine is equivalent).
      2. the exit drain: split its per-proc-lane waits across a drain chain
         (all still complete before the barrier + sem clear).
    """
    import concourse.tile as _tile
    import concourse.mybir as mybir
    from concourse.vector_clock import ScopedClock

    if getattr(_tile.TileContext, "_demosaic_drain_patch", False):
        return

    _orig_commit_and_lower = _tile.TileContext._commit_and_lower

    def _commit_and_lower(self, inst, original_block, old_bb_map, bb_to_exit_bb):
        si = getattr(inst, "sync_info", None)
        eng = getattr(inst, "engine", None)
        if (
            si is not None
            and si.on_wait
            and len(si.on_wait) > 1
            and eng is not None
            and eng in self.nc.engines
        ):
            waits = list(si.on_wait)
            for w in waits[:-1]:
                nop = self.nc.engines[eng].nop(nofuse=True, hint="waitsplit")
                nop.ins.sync_info = mybir.SyncInfo(on_wait=[w], on_update=[])
            inst.sync_info = mybir.SyncInfo(
                on_wait=waits[-1:], on_update=list(si.on_update or [])
            )
        return _orig_commit_and_lower(
            self, inst, original_block, old_bb_map, bb_to_exit_bb
        )

    def _drain_and_barrier(self, tick_clock, wait_clock):
        nc = self.nc
        drain_inst = nc.sync.drain()
        wait_clock.add_sem_waits(
            drain_inst.ins, ScopedClock({None: tick_clock.global_clock})
        )
        si = drain_inst.ins.sync_info
        waits = list(si.on_wait) if si and si.on_wait else []
        if len(waits) > 1:
            si.on_wait = waits[:1]
            for i in range(1, len(waits)):
                extra = nc.sync.drain()
                extra.ins.sync_info = mybir.SyncInfo(
                    on_wait=waits[i:i + 1], on_update=[]
                )
        nc.all_engine_barrier()
        assert self.sems is not None
        popped = nc._tile_sem_poison_stack.pop()
        assert popped is self._sem_poison
        nc.clear_and_free_semaphores(list(self.sems.allocated().values()))
        nc.all_engine_barrier()

    _tile.TileContext._commit_and_lower = _commit_and_lower
    _tile.TileContext._drain_and_barrier = _drain_and_barrier
    _tile.TileContext._demosaic_drain_patch = True


def _build_bass():
    _patch_tile_drain()
    import concourse.bass as bass
    import concourse.mybir as mybir
    import concourse.tile as tile

    f32 = mybir.dt.float32
    f16 = mybir.dt.bfloat16
    Alu = mybir.AluOpType
    Act = mybir.ActivationFunctionType

    nc = bass.Bass()
    # input: [xe | xo] deinterleaved planes, 1026 cols each
    xp = nc.dram_tensor("xp", (_HH + 4, _PW), f16, kind="ExternalInput")
    # compact conv-site output, full blocks: [block, 128 rows, (pp,pack)*1024]
    qout = nc.dram_tensor("qout", (8, 128, 4 * (_W // 2)), f16,
                          kind="ExternalOutput")
    # tail block: 4 quarter-fields x 16 rows packed in 64 partitions, pp-major
    qtail = nc.dram_tensor("qtail", (2, 64, _W // 2), f16, kind="ExternalOutput")
    wdram = nc.inline_tensor(_build_wmats(), name="wmats")

    XO = _PLANE  # xo plane base column in the packed tile

    with tile.TileContext(nc) as tc:
        with (
            tc.tile_pool(name="wpool", bufs=1) as wpool,
            tc.tile_pool(name="xpool", bufs=3) as xpool,
            tc.tile_pool(name="uwpool", bufs=3) as uwpool,
            tc.tile_pool(name="qpool", bufs=2) as qpool,
            tc.tile_pool(name="psum", bufs=4, space="PSUM") as psum_pool,
        ):
            wt = wpool.tile([128, 18 * 128], f16)

            for b in range(_NBLK):
                s0 = 124 * b if b < 8 else (_HH - _BLK_OUT)  # block start (padded row)

                xt = xpool.tile([128, _PW], f16)
                nc.sync.dma_start(xt[:], xp[s0:s0 + 128, :])
                if b == 0:
                    # W load issued AFTER block 0's input so the first matmul
                    # isn't queued behind it on the DMA ring
                    nc.sync.dma_start(wt[:], wdram[:])

                # movers, half-res col j in 0..1023 (full-res J = 2j+p+2):
                #   p=0: center xe[j+1];  u0 = xo[j]+xo[j+1];  w0 = xe[j]+xe[j+2]
                #   p=1: center xo[j+1];  u1 = xe[j+1]+xe[j+2]; w1 = xo[j]+xo[j+2]
                # u operands include an odd element offset (VectorE faults) ->
                # GpSimd; w operands are 4B-aligned -> VectorE. Separate tiles
                # per mover keep producer->consumer dependencies narrow (a
                # fused tile serializes the first matmul behind all four TTs).
                u0 = uwpool.tile([128, 1024], f16, name=f"u0_b{b}", tag="u0")
                u1 = uwpool.tile([128, 1024], f16, name=f"u1_b{b}", tag="u1")
                w0 = uwpool.tile([128, 1024], f16, name=f"w0_b{b}", tag="w0")
                w1 = uwpool.tile([128, 1024], f16, name=f"w1_b{b}", tag="w1")
                nc.gpsimd.tensor_tensor(
                    u0[:], xt[:, XO:XO + 1024], xt[:, XO + 1:XO + 1025], Alu.add)
                nc.vector.tensor_tensor(
                    w0[:], xt[:, 0:1024], xt[:, 2:1026], Alu.add)
                nc.gpsimd.tensor_tensor(
                    u1[:], xt[:, 1:1025], xt[:, 2:1026], Alu.add)
                nc.vector.tensor_tensor(
                    w1[:], xt[:, XO:XO + 1024], xt[:, XO + 2:XO + 1026], Alu.add)

                movers = {
                    (0, 0): xt[:, 1:1025],
                    (0, 1): u0[:],
                    (0, 2): w0[:],
                    (1, 0): xt[:, XO + 1:XO + 1025],
                    (1, 1): u1[:],
                    (1, 2): w1[:],
                }

                if b < 8:
                    for pp in range(2):
                        for pack in range(2):
                            qt = qpool.tile([128, 1024], f16,
                                            name=f"qt{pp}{pack}_b{b}", tag=f"q{pp}{pack}")
                            ps = psum_pool.tile([128, 1024], f32,
                                                name=f"ps_b{b}p{pp}{pack}", tag="ps")
                            for vi in range(3):
                                mv = movers[(pp, vi)]
                                idx = (pp * 2 + pack) * 3 + vi
                                for c in range(_NCHUNK):
                                    nc.tensor.matmul(
                                        ps[:, 512 * c:512 * c + 512],
                                        wt[:, idx * 128:(idx + 1) * 128],
                                        mv[:, 512 * c:512 * c + 512],
                                        start=(vi == 0),
                                        stop=(vi == 2),
                                    )
                            # all evacuations on ACT: PSUM reads don't contend
                            # with the PE's SBUF moving-tensor stream (SBUF-
                            # sourced vector ops run ~3x slower under PE load).
                            # Relu only; the host applies min(.,1).
                            e = (pp * 2 + pack) * 1024
                            nc.scalar.activation(qt[:], ps[:], Act.Relu, bias=0.0)
                            nc.sync.dma_start(qout[b, :, e:e + 1024], qt[:])
                else:
                    # tail block: the host only consumes quarter-rows 496..511
                    # (m=46..61); a packed W computes all four (pack, slot)
                    # fields x 16 rows in 64 output partitions -> 12 matmuls
                    # instead of 24 and a shorter post-stream drain
                    for pp in range(2):
                        qt = qpool.tile([64, 1024], f16, name=f"qtail{pp}",
                                        tag=f"qtl{pp}")
                        ps = psum_pool.tile([128, 1024], f32, name=f"ps_tp{pp}",
                                            tag="ps")
                        for vi in range(3):
                            mv = movers[(pp, vi)]
                            idx = 12 + pp * 3 + vi
                            for c in range(_NCHUNK):
                                nc.tensor.matmul(
                                    ps[0:64, 512 * c:512 * c + 512],
                                    wt[:, idx * 128:idx * 128 + 64],
                                    mv[:, 512 * c:512 * c + 512],
                                    start=(vi == 0),
                                    stop=(vi == 2),
                                )
                        nc.scalar.activation(qt[:], ps[0:64, 0:1024], Act.Relu,
                                             bias=0.0)
                        nc.sync.dma_start(qtail[pp], qt[:])

    return nc


_nc_cache = None

# set by an external harness to capture an NTFF profile; harmless otherwise
TRACE = False
LAST_EXEC_NS = None
LAST_RESULT = None


def _get_nc():
    global _nc_cache
    if _nc_cache is None:
        _nc_cache = _build_bass()
    return _nc_cache


def kernel(x: np.ndarray, k: np.ndarray | None = None) -> np.ndarray:
    """x: (4, 1, 2048, 2048) float32 RGGB mosaic -> (4, 3, 2048, 2048) float32."""
    from concourse.bass_utils import run_bass_kernel_spmd

    x = np.asarray(x, dtype=np.float32)
    assert x.shape == (_N, 1, _H, _W), x.shape
    import ml_dtypes
    xpad = np.pad(x[:, 0], ((0, 0), (2, 2), (2, 2)), mode="reflect")
    # deinterleave columns: [xe | xo] planes, then bf16
    xdi = np.concatenate([xpad[:, :, 0::2], xpad[:, :, 1::2]], axis=2)
    xdi = xdi.astype(ml_dtypes.bfloat16)

    in_maps = []
    for core in range(_NCORES):
        img, half = divmod(core, 2)
        h0 = half * _HH
        in_maps.append({"xp": np.ascontiguousarray(xdi[img, h0:h0 + _HH + 4, :])})

    nc = _get_nc()
    res = run_bass_kernel_spmd(nc, in_maps, list(range(_NCORES)), trace=TRACE)
    global LAST_EXEC_NS, LAST_RESULT
    LAST_EXEC_NS = res.exec_time_ns
    LAST_RESULT = res

    outp = np.empty((_N, 3, _H, _W), dtype=np.float32)
    for core in range(_NCORES):
        img, half = divmod(core, 2)
        h0 = half * _HH
        xs = x[img, 0, h0:h0 + _HH, :]
        o = outp[img, :, h0:h0 + _HH, :]
        # copy sites straight from the mosaic (exact, already in [0,1))
        o[0, 0::2, 0::2] = xs[0::2, 0::2]
        o[1, 0::2, 1::2] = xs[0::2, 1::2]
        o[1, 1::2, 0::2] = xs[1::2, 0::2]
        o[2, 1::2, 1::2] = xs[1::2, 1::2]
        # conv sites from the device's compact quarter-fields
        q = res.results[core]["qout"]  # (8, 128, 4096) fp16
        qt = res.results[core]["qtail"]  # (2, 64, 1024) fp16
        for (pp, pack, slot), (ch, ra, _ki) in _QF_MAP.items():
            Q = np.empty((_HH // 2, _W // 2), dtype=np.float32)
            e = (pp * 2 + pack) * 1024
            rows = q[:, 64 * slot:64 * slot + 64, e:e + 1024]
            for b in range(8):
                Q[62 * b:62 * b + 62] = rows[b, 0:62]
            g = 16 * (2 * pack + slot)
            Q[496:512] = qt[pp, g:g + 16]
            o[ch, ra::2, pp::2] = np.minimum(Q, 1.0)
    return outp
